# revision 1
# baseline (speedup 1.0000x reference)
"""Trainium2 Bass kernel for nn_ActionNetwork (gnn_message_passing).

Strategy (pure data parallel over the episode axis, 8 cores):
  - Host precomputes tiny fused matrices: the whole linear front-end
    (per-node W0 gather + W1 mix + pairwise potential difference + the
    distribute_param row scale) collapses into one (88 -> 64) matmul
    "dmat" plus a per-pair bias "dconst".
  - On chip, episodes live on SBUF partitions ("natural" layout).  The
    only transposed stage is the PE matmul computing
    relu(dp*(pot_i-pot_j)+dconst); its input/output cross PSUM<->SBUF
    via ACT/DVE copies fused with the relu.
  - Everything else (val/action/price pipeline) is elementwise DVE/ACT/
    GPSIMD work with stride-0 broadcast access patterns, so the output
    is produced directly in episode-major order and DMA'd out with no
    transposes.
"""

import numpy as np

import concourse.bass as bass
import concourse.tile as tile
from concourse import bacc, mybir
from concourse.bass_utils import run_bass_kernel_spmd

F32 = mybir.dt.float32
BF16 = mybir.dt.bfloat16
ALU = mybir.AluOpType
ACTF = mybir.ActivationFunctionType

N = 8
MINI = 2
EP = 131072
F = 88
FI = 89
NCORES = 8
EPC = EP // NCORES          # 16384 episodes per core
BLK = 16                    # 128-episode blocks per btile
BT = 128 * BLK              # 2048 episodes per btile
NBT = EPC // BT             # 8 btiles per core

_CACHE = {}


def _kernel_body(tc, outc, xc, xhi, xlo, dmat_d, iden_d, bench_reps=None, stages='full'):
    nc = tc.nc
    from contextlib import ExitStack
    ctx = ExitStack()
    with ctx:
        const_pool = ctx.enter_context(tc.tile_pool(name="const", bufs=1))
        xin_pool = ctx.enter_context(tc.tile_pool(name="xin", bufs=3))
        xt_pool = ctx.enter_context(tc.tile_pool(name="xt", bufs=6))
        pair_pool = ctx.enter_context(tc.tile_pool(name="pair", bufs=3))
        tail_pool = ctx.enter_context(tc.tile_pool(name="tail", bufs=3))
        node_pool = ctx.enter_context(tc.tile_pool(name="node", bufs=3))
        stag_pool = ctx.enter_context(tc.tile_pool(name="stag", bufs=3))
        ps_xt = ctx.enter_context(tc.tile_pool(name="ps_xt", bufs=2, space="PSUM"))
        ps_xl = ctx.enter_context(tc.tile_pool(name="ps_xl", bufs=2, space="PSUM"))
        ps_dn = ctx.enter_context(tc.tile_pool(name="ps_dn", bufs=2, space="PSUM"))
        ps_qq = ctx.enter_context(tc.tile_pool(name="ps_qq", bufs=1, space="PSUM"))

        # constants (dmat has the per-pair bias folded in as row 88; the
        # host appends a ones column to x to activate it).  Loaded on the
        # ACT HWDGE ring so they don't queue behind the episode loads.
        iden_t = const_pool.tile([128, 128], F32, tag="iden")
        nc.scalar.dma_start(iden_t[:], iden_d)
        c04_t = const_pool.tile([128, 1], F32, tag="c04")
        nc.vector.memset(c04_t[:], 0.4)
        dqh_t = const_pool.tile([FI, 128], BF16, tag="dqh")
        nc.scalar.dma_start(dqh_t[:], dmat_d[0:FI])
        dql_t = const_pool.tile([FI, 128], BF16, tag="dql")
        nc.scalar.dma_start(dql_t[:], dmat_d[FI:2 * FI])
        idenb_t = const_pool.tile([128, 128], BF16, tag="idenb")
        nc.scalar.copy(idenb_t[:], iden_t[:])

        # episode -> (partition, chunk) mapping: partition p owns the 16
        # consecutive episodes [2048*bb + 16p, +16); chunk k picks the k-th.
        # This makes every DMA run 16 rows contiguous in DRAM (line rate);
        # the whole pipeline is per-episode so the permutation is harmless.
        xc_r = xc.rearrange("(bb p k) f -> bb p k f", p=128, k=BLK)
        xhi_r = xhi.rearrange("(bb p k) f -> bb p k f", p=128, k=BLK)
        xlo_r = xlo.rearrange("(bb p k) f -> bb p k f", p=128, k=BLK)
        out_r = outc.rearrange("(bb p k) o -> bb p k o", p=128, k=BLK)

        def bc(node_ap3):
            # (128, BLK, N) node tensor -> broadcast over trailing pair dim
            return node_ap3.unsqueeze(3).broadcast_to((128, BLK, N, N))

        if bench_reps is not None:
            loop_cm = tc.For_i(
                0, bench_reps, 1,
                hint_engines=(mybir.EngineType.PE, mybir.EngineType.DVE,
                              mybir.EngineType.Activation),
            )
            ctx.enter_context(loop_cm)

        for b in range(NBT):
            xin = xin_pool.tile([128, BLK * FI], F32, tag="xin")
            xin3 = xin[:].rearrange("p (k f) -> p k f", f=FI)
            if b == 0:
                # fine-grained first load so the PE front-end starts early
                for c in range(BLK // 4):
                    nc.sync.dma_start(
                        xin3[:, 4 * c:4 * c + 4, :], xc_r[b][:, 4 * c:4 * c + 4, :]
                    )
            else:
                nc.sync.dma_start(xin3, xc_r[b])
            xinhi = xin_pool.tile([128, BLK * FI], BF16, tag="xinhi")
            xinhi3 = xinhi[:].rearrange("p (k f) -> p k f", f=FI)
            nc.sync.dma_start(xinhi3, xhi_r[b])
            xinlo = xin_pool.tile([128, BLK * FI], BF16, tag="xinlo")
            xinlo3 = xinlo[:].rearrange("p (k f) -> p k f", f=FI)
            nc.sync.dma_start(xinlo3, xlo_r[b])

            # transpose 128-episode chunks, then contract features with
            # the chunk as the stationary operand -> diff lands episode-
            # major in PSUM directly (no output transposes needed)
            va = pair_pool.tile([128, BLK * 64], F32, tag="va")
            qqn = ps_qq.tile([128, BLK * 64], F32, tag="qqn")
            for c in range(BLK // 4):
                xtA = ps_xt.tile([FI, 512], BF16, tag="xtA")
                xtAl = ps_xl.tile([FI, 512], BF16, tag="xtAl")
                for kk in range(4):
                    k = 4 * c + kk
                    nc.tensor.transpose(
                        xtA[:, 128 * kk:128 * kk + 128], xinhi3[:, k, :],
                        idenb_t[:],
                    )
                    nc.tensor.transpose(
                        xtAl[:, 128 * kk:128 * kk + 128], xinlo3[:, k, :],
                        idenb_t[:],
                    )
                xt_c = xt_pool.tile([FI, 512], BF16, tag="xt")
                nc.scalar.copy(xt_c[:], xtA[:])
                xt_l = xt_pool.tile([FI, 512], BF16, tag="xtl")
                nc.scalar.copy(xt_l[:], xtAl[:])
                diffnat = ps_dn.tile([128, 256], F32, tag="diffnat")
                for kk in range(4):
                    k = 4 * c + kk
                    xck = xt_c[:, 128 * kk:128 * kk + 128]
                    xlk = xt_l[:, 128 * kk:128 * kk + 128]
                    dnk = diffnat[:, 64 * kk:64 * kk + 64]
                    qqk = qqn[:, 64 * k:64 * k + 64]
                    nc.tensor.matmul(dnk, xck, dqh_t[:, 0:64],
                                     start=True, stop=False)
                    nc.tensor.matmul(dnk, xck, dql_t[:, 0:64],
                                     start=False, stop=False)
                    nc.tensor.matmul(qqk, xck, dqh_t[:, 64:128],
                                     start=True, stop=False)
                    nc.tensor.matmul(qqk, xck, dql_t[:, 64:128],
                                     start=False, stop=False)
                    nc.tensor.matmul(qqk, xlk, dqh_t[:, 64:128],
                                     start=False, stop=True)
                    nc.tensor.matmul(dnk, xlk, dqh_t[:, 0:64],
                                     start=False, stop=True)
                # va = relu(dp*diff + dconst), psum -> sbuf per c-chunk
                nc.scalar.activation(
                    va[:, 256 * c:256 * c + 256], diffnat[:], ACTF.Relu
                )

            if stages == 'front':
                vaf = va[:].rearrange("p (k d) -> p k d", d=64)
                nc.sync.dma_start(out_r[b][:, :, 0:64], vaf)
                continue

            # ---- natural-layout pair/node pipeline ----
            queue4 = xin3[:, :, 24:88].rearrange("p k (a b) -> p k a b", b=N)
            veh3 = xin3[:, :, 0:8]

            # val = relu(dp*diff + dconst) + qp*queue
            val = pair_pool.tile([128, BLK * 64], F32, tag="val")
            val4 = val[:].rearrange("p (k a b) -> p k a b", a=N, b=N)
            nc.vector.tensor_add(val[:], va[:], qqn[:])

            rs = node_pool.tile([128, BLK * N], F32, tag="rs")
            rs3 = rs[:].rearrange("p (k i) -> p k i", i=N)
            nc.vector.tensor_reduce(rs3, val4, axis=mybir.AxisListType.X, op=ALU.add)

            remain = node_pool.tile([128, BLK * N], F32, tag="remain")
            remain3 = remain[:].rearrange("p (k i) -> p k i", i=N)
            nc.vector.tensor_sub(remain3, veh3, rs3)

            denom = node_pool.tile([128, BLK * N], F32, tag="denom")
            nc.vector.scalar_tensor_tensor(
                denom[:], remain[:], 0.0, rs[:], op0=ALU.max, op1=ALU.add
            )
            rden = node_pool.tile([128, BLK * N], F32, tag="rden")
            rden3 = rden[:].rearrange("p (k i) -> p k i", i=N)
            nc.vector.reciprocal(rden[:], denom[:])

            rv = node_pool.tile([128, BLK * N], F32, tag="rv")
            rv3 = rv[:].rearrange("p (k i) -> p k i", i=N)
            nc.gpsimd.tensor_mul(rv3, veh3, rden3)

            s_t = node_pool.tile([128, BLK * N], F32, tag="s_t")
            s3 = s_t[:].rearrange("p (k i) -> p k i", i=N)
            nc.scalar.activation(s_t[:], rv[:], ACTF.Relu, bias=1.0, scale=-1.0)

            t_diag = node_pool.tile([128, BLK * N], F32, tag="t_diag")
            t_diag3 = t_diag[:].rearrange("p (k i) -> p k i", i=N)
            nc.vector.scalar_tensor_tensor(
                t_diag[:], remain[:], 0.0, rden[:], op0=ALU.max, op1=ALU.mult
            )

            dep = node_pool.tile([128, BLK * N], F32, tag="dep")
            nc.gpsimd.tensor_mul(dep[:], rv[:], rs[:])

            m_t = node_pool.tile([128, BLK * N], F32, tag="m_t")
            m3 = m_t[:].rearrange("p (k i) -> p k i", i=N)
            nc.gpsimd.tensor_add(m3, xin3[:, :, 8:24:2], xin3[:, :, 9:24:2])

            raw = pair_pool.tile([128, BLK * 64], F32, tag="raw")
            raw4 = raw[:].rearrange("p (k a b) -> p k a b", a=N, b=N)
            nc.vector.tensor_mul(raw4, val4, bc(rv3))

            stag = stag_pool.tile([128, BLK * 128], F32, tag="stag")
            stag5 = stag[:].rearrange("p (k i c) -> p k i c", i=N, c=2 * N)
            stag3 = stag[:].rearrange("p (k d) -> p k d", d=2 * N * N)
            # action off-diagonal (diag positions get 0*rden=0, fixed below)
            nc.vector.tensor_mul(stag5[:, :, :, 0:8], val4, bc(rden3))
            # action diagonal = relu(remain)/denom at column 17*i
            nc.gpsimd.tensor_copy(stag3[:, :, 0:121:17], t_diag3)

            fg = pair_pool.tile([128, BLK * 64], F32, tag="fg")
            fg4 = fg[:].rearrange("p (k a b) -> p k a b", a=N, b=N)
            nc.gpsimd.tensor_mul(fg4, val4, bc(s3))

            t1 = pair_pool.tile([128, BLK * 64], F32, tag="t1")
            t14 = t1[:].rearrange("p (k a b) -> p k a b", a=N, b=N)
            nc.vector.tensor_sub(t14, queue4, raw4)

            # fq = relu(t1) on Pool; A = fq - fg
            a_t = tail_pool.tile([128, BLK * 64], F32, tag="a_t")
            a4 = a_t[:].rearrange("p (k a b) -> p k a b", a=N, b=N)
            nc.vector.scalar_tensor_tensor(
                a_t[:], t1[:], 0.0, fg[:], op0=ALU.max, op1=ALU.subtract
            )

            arr = node_pool.tile([128, BLK * N], F32, tag="arr")
            arr3 = arr[:].rearrange("p (k j) -> p k j", j=N)
            raw_perm = raw[:].rearrange("p (k i j) -> p k j i", i=N, j=N)
            nc.vector.tensor_reduce(arr3, raw_perm, axis=mybir.AxisListType.X, op=ALU.add)

            z1 = node_pool.tile([128, BLK * N], F32, tag="z1")
            nc.gpsimd.tensor_sub(z1[:], arr[:], dep[:])
            z2 = node_pool.tile([128, BLK * N], F32, tag="z2")
            nc.gpsimd.tensor_add(z2[:], veh3, m3)
            fv = node_pool.tile([128, BLK * N], F32, tag="fv")
            nc.gpsimd.tensor_add(fv[:], z1[:], z2[:])

            ints = node_pool.tile([128, BLK * N], F32, tag="ints")
            nc.gpsimd.tensor_mul(ints[:], s_t[:], rs[:])
            t5 = node_pool.tile([128, BLK * N], F32, tag="t5")
            nc.gpsimd.tensor_sub(t5[:], fv[:], ints[:])
            r2b = node_pool.tile([128, BLK * N], F32, tag="r2b")
            r2b3 = r2b[:].rearrange("p (k i) -> p k i", i=N)
            nc.scalar.activation(r2b[:], t5[:], ACTF.Relu, bias=0.0, scale=1.0 / (N - 1))

            # intention = -A + r2b_bcast
            intn = tail_pool.tile([128, BLK * 64], F32, tag="intn")
            nc.vector.scalar_tensor_tensor(
                intn[:].rearrange("p (k a b) -> p k a b", a=N, b=N),
                a4, -1.0, bc(r2b3), op0=ALU.mult, op1=ALU.add,
            )

            # price = relu(0.4 - 0.25*relu(z)) + 0.6 on the (idle) ACT engine
            u_t = tail_pool.tile([128, BLK * 64], F32, tag="u_t")
            nc.scalar.activation(u_t[:], intn[:], ACTF.Relu)
            v_t = tail_pool.tile([128, BLK * 64], F32, tag="v_t")
            nc.scalar.activation(
                v_t[:], u_t[:], ACTF.Relu, bias=c04_t[:, 0:1], scale=-0.25
            )
            nc.scalar.activation(
                stag5[:, :, :, 8:16],
                v_t[:].rearrange("p (k a b) -> p k a b", a=N, b=N),
                ACTF.Copy, bias=0.6,
            )

            half = BLK // 2
            nc.sync.dma_start(out_r[b][:, 0:half, :], stag3[:, 0:half, :])
            nc.sync.dma_start(out_r[b][:, half:BLK, :], stag3[:, half:BLK, :])


def _build(bench_reps=None, stages='full'):
    nc = bacc.Bacc(
        "TRN2", target_bir_lowering=False, debug=False,
        enable_asserts=False, num_devices=NCORES,
    )
    xc = nc.dram_tensor("xc", [EPC, FI], F32, kind="ExternalInput").ap()
    xhi = nc.dram_tensor("xhi", [EPC, FI], BF16, kind="ExternalInput").ap()
    xlo = nc.dram_tensor("xlo", [EPC, FI], BF16, kind="ExternalInput").ap()
    dmat_d = nc.dram_tensor("dmat", [2 * FI, 128], BF16, kind="ExternalInput").ap()
    iden_d = nc.dram_tensor("iden", [128, 128], F32, kind="ExternalInput").ap()
    outc = nc.dram_tensor("outc", [EPC, 2 * N * N], F32, kind="ExternalOutput").ap()
    with tile.TileContext(nc) as tc:
        _kernel_body(tc, outc, xc, xhi, xlo, dmat_d, iden_d,
                     bench_reps=bench_reps, stages=stages)
    nc.compile()
    return nc


def _host_consts(W0, b0, W1, b1, dp, qp):
    n = np.arange(N)
    A0 = np.zeros((N, F), np.float32)
    A0[n, n] += W0[:, 0]
    for i in range(MINI):
        A0[n, N + N * n + i] += W0[:, 1 + i]
    for j in range(N):
        A0[n, 24 + N * n + j] += W0[:, 3 + j]
        A0[n, 24 + N * j + n] += W0[:, 11 + j]
    A1 = (W1 @ A0).astype(np.float32)
    c1 = (W1 @ b0 + b1).astype(np.float32)
    DM = (dp[:, :, None] * (A1[:, None, :] - A1[None, :, :])).astype(np.float32)
    dconst = (dp * (c1[:, None] - c1[None, :])).astype(np.float32).reshape(64)
    dmat = np.zeros((FI, 64), np.float32)
    dmat[:F, :] = DM.reshape(64, F).T
    dmat[F, :] = dconst                     # bias row, driven by ones column
    qpf = qp.astype(np.float32).copy()
    np.fill_diagonal(qpf, 0.0)
    qmat = np.zeros((FI, 64), np.float32)
    for i in range(N):
        for j in range(N):
            qmat[24 + N * i + j, N * i + j] = qpf[i, j]
    import ml_dtypes
    dqmat = np.concatenate([dmat, qmat], axis=1)    # (89, 128) f32
    hi = dqmat.astype(ml_dtypes.bfloat16)
    lo = (dqmat - hi.astype(np.float32)).astype(ml_dtypes.bfloat16)
    dq2 = np.concatenate([hi, lo], axis=0)          # (178, 128) bf16
    iden = np.eye(128, dtype=np.float32)
    return np.ascontiguousarray(dq2), iden


def kernel(x, W0, b0, W1, b1, distribute_param, queue_param, _trace=False):
    x = np.ascontiguousarray(np.asarray(x, np.float32))
    W0 = np.asarray(W0, np.float32)
    b0 = np.asarray(b0, np.float32)
    W1 = np.asarray(W1, np.float32)
    b1 = np.asarray(b1, np.float32)
    dp = np.asarray(distribute_param, np.float32)
    qp = np.asarray(queue_param, np.float32)

    if "nc" not in _CACHE:
        _CACHE["nc"] = _build()
    nc = _CACHE["nc"]

    import ml_dtypes
    dqmat, iden = _host_consts(W0, b0, W1, b1, dp, qp)
    xi = np.empty((EP, FI), np.float32)
    xi[:, :F] = x
    xi[:, F] = 1.0
    xhi = xi.astype(ml_dtypes.bfloat16)
    xlo = (xi - xhi.astype(np.float32)).astype(ml_dtypes.bfloat16)
    x8 = xi.reshape(NCORES, EPC, FI)
    xhi8 = xhi.reshape(NCORES, EPC, FI)
    xlo8 = xlo.reshape(NCORES, EPC, FI)
    in_maps = [
        {"xc": np.ascontiguousarray(x8[c]),
         "xhi": np.ascontiguousarray(xhi8[c]),
         "xlo": np.ascontiguousarray(xlo8[c]),
         "dmat": dqmat, "iden": iden}
        for c in range(NCORES)
    ]
    res = run_bass_kernel_spmd(
        nc, in_maps, core_ids=list(range(NCORES)), trace=_trace
    )
    out = np.concatenate([res.results[c]["outc"] for c in range(NCORES)], axis=0)
    if _trace:
        _CACHE["last_results"] = res
    return out

